# revision 10
# baseline (speedup 1.0000x reference)
"""Bilinear grid-sample (align_corners=True, border-clamped) on Trainium2.

Problem: x [8, 64, 256, 256] f32, grid [8, 256, 256, 2] f32 in [-1, 1]
         -> out [8, 64, 256, 256] f32.

Sharding: pure data-parallel over batch N=8 -> one sample per NeuronCore.

Per-core layout strategy (host-side marshalling only; all arithmetic that
is part of the operator runs on-device):
  - x[n] is fed channels-last as xt [H*W + 2W, C] f32 (zero pad rows), so
    one 512B contiguous chunk = the (x0, x0+1) pixel pair for all C chans.
  - gathers use the GPSIMD dma_gather custom DMA (int16 indices, 16-bit
    sign limit => 32768 addressable 256B rows = half the image). Pixels
    are therefore binned by their y0 image half (top: y0 < H/2, bottom:
    y0 >= H/2). One int16 relative row index rel = y0*W + x0 - (half ?
    H*W/2 : 0) serves all four gathers (half x {row y0, row y1}): the
    four variants differ only in the static base offset of the source
    access pattern.
  - the host bins pixel ids into [top..., pad, bottom..., pad] where pads
    are synthetic corner pixels; every gather call (slot) is homogeneous
    in half, so its base is static. Binning uses the same fp32 operations
    the device uses, and the device clamps rel to [0, 32767] anyway.
  - the device computes indices/weights, gathers, combines, and stores
    results in stream order; the host permutes rows back and transposes
    to [C, H, W].

dma_gather stream mapping (hardware-verified): stream position i takes
its int16 index from idx_tile[i % 16, i // 16] (partitions 0-15, and the
replicas at partitions 16-31 for queue 0's second Q7 core) and writes the
gathered row to partition i % 128, slot i // 128. Partition p therefore
owns stream positions == p (mod 128); the grid is fed in that layout, and
a small DRAM bounce rearranges the computed rel16 into the 16-partition
wrapped layout.
"""

import numpy as np

import concourse.bacc as bacc
import concourse.bass as bass
import concourse.tile as tile
from concourse import bass_utils, mybir

N, C, H, W = 8, 64, 256, 256

F32 = mybir.dt.float32
I32 = mybir.dt.int32
I16 = mybir.dt.int16
AF = mybir.ActivationFunctionType
OP = mybir.AluOpType
P = 128


def build_program(
    h=H, w=W, c=C, k=32, sA=9, sB=9, gbufs=2, mbufs=2, rbufs=3, repeat=1
):
    """Per-core Bass program for sA top-half + sB bottom-half gather slots.

    Each slot covers 128*k stream positions; gathers fetch elem 2*c floats
    (512B) at stride c floats (256B) from the padded channels-last slab.
    repeat > 1 re-emits the whole pipeline for repeat-delta timing.
    """
    npix = h * w
    nrows = npix + 2 * w
    halfbase = (h // 2) * w          # pixel-row offset of the bottom half
    S = sA + sB
    qp = S * k                        # stream positions per partition
    scx = (w - 1) / 2.0
    scy = (h - 1) / 2.0
    relmax = float(halfbase + w - 1)  # clamp bound; == 32767 at full size

    nc = bacc.Bacc(trn_type="TRN2", debug=False)
    xt = nc.dram_tensor("xt", [nrows, c], F32, kind="ExternalInput")
    gridf = nc.dram_tensor("gridf", [P, qp * 2], F32, kind="ExternalInput")
    bounces = [
        nc.dram_tensor(f"bounce{r}", [P * qp], I16, kind="Internal")
        for r in range(repeat)
    ]
    outdev = nc.dram_tensor("outdev", [P, qp * c], F32, kind="ExternalOutput")

    # overlapping pixel-pair views of the slab, one per (half, y-row) base
    def pair_view(base_pixrow):
        cnt = nrows - base_pixrow - 2
        return bass.AP(xt, base_pixrow * c, [[c, cnt], [1, 2 * c]])

    bases = {
        "A0": pair_view(0),
        "A1": pair_view(w),
        "B0": pair_view(halfbase),
        "B1": pair_view(halfbase + w),
    }

    with tile.TileContext(nc) as tc:
      for rep in range(repeat):
        with tc.tile_pool(name=f"persist{rep}", bufs=1) as persist:
            wA = persist.tile([P, qp * 2], F32, tag="wA")   # (w00, w01)
            wB = persist.tile([P, qp * 2], F32, tag="wB")   # (w10, w11)
            wrapped = persist.tile([P, qp * 8], I16, tag="wrapped")

            with tc.tile_pool(name=f"prep{rep}", bufs=1) as prep:
                gridt = prep.tile([P, qp * 2], F32, tag="gridt")
                nc.sync.dma_start(gridt[:], gridf[:])
                g3 = gridt[:].rearrange("p (q xy) -> p q xy", xy=2)

                def sc_mul_add(src, scale, name):
                    # (src + 1) * scale with the reference's rounding order,
                    # as two explicitly rounded DVE ops (host replicates it)
                    a = prep.tile([P, qp], F32, tag=f"sma_{name}")
                    nc.vector.tensor_scalar_add(a[:], src, 1.0)
                    nc.vector.tensor_scalar_mul(a[:], a[:], scale)
                    return a

                ix = sc_mul_add(g3[:, :, 0], scx, "x")
                iy = sc_mul_add(g3[:, :, 1], scy, "y")

                def floor_and_fracs(v, name):
                    vi = prep.tile([P, qp], I32, tag=f"vi_{name}")
                    nc.vector.tensor_copy(vi[:], v)
                    vf = prep.tile([P, qp], F32, tag=f"vf_{name}")
                    nc.vector.tensor_copy(vf[:], vi[:])
                    m = prep.tile([P, qp], F32, tag=f"m_{name}")
                    nc.vector.tensor_tensor(m[:], vf[:], v, op=OP.is_gt)
                    nc.vector.tensor_tensor(vf[:], vf[:], m[:], op=OP.subtract)
                    fr1 = prep.tile([P, qp], F32, tag=f"fr1_{name}")
                    nc.vector.tensor_tensor(fr1[:], v, vf[:], op=OP.subtract)
                    fr0 = prep.tile([P, qp], F32, tag=f"fr0_{name}")
                    nc.scalar.activation(fr0[:], fr1[:], AF.Copy, bias=1.0, scale=-1.0)
                    return vf, fr0, fr1

                x0f, wx0, wx1 = floor_and_fracs(ix[:], "x")
                y0f, wy0, wy1 = floor_and_fracs(iy[:], "y")

                wA2 = wA[:].rearrange("p (q x) -> p q x", x=2)
                wB2 = wB[:].rearrange("p (q x) -> p q x", x=2)
                nc.vector.tensor_tensor(wA2[:, :, 0], wy0[:], wx0[:], op=OP.mult)
                nc.vector.tensor_tensor(wA2[:, :, 1], wy0[:], wx1[:], op=OP.mult)
                nc.vector.tensor_tensor(wB2[:, :, 0], wy1[:], wx0[:], op=OP.mult)
                nc.vector.tensor_tensor(wB2[:, :, 1], wy1[:], wx1[:], op=OP.mult)

                # rel = y0*w + x0 - halfbase*(y0 >= h/2), clamped to int16 range
                mhalf = prep.tile([P, qp], F32, tag="mhalf")
                nc.vector.tensor_scalar(
                    mhalf[:], y0f[:], float(h // 2), None, op0=OP.is_ge
                )
                idxf = prep.tile([P, qp], F32, tag="idxf")
                nc.vector.scalar_tensor_tensor(
                    idxf[:], in0=y0f[:], scalar=float(w), in1=x0f[:],
                    op0=OP.mult, op1=OP.add,
                )
                relf = prep.tile([P, qp], F32, tag="relf")
                nc.vector.scalar_tensor_tensor(
                    relf[:], in0=mhalf[:], scalar=-float(halfbase), in1=idxf[:],
                    op0=OP.mult, op1=OP.add,
                )
                nc.vector.tensor_scalar_max(relf[:], relf[:], 0.0)
                nc.vector.tensor_scalar_min(relf[:], relf[:], relmax)
                rel16 = prep.tile([P, qp], I16, tag="rel16")
                nc.vector.tensor_copy(rel16[:], relf[:])

                # bounce through DRAM into the 16-partition wrapped layout,
                # replicated to partition groups 0 and 1 (queue 0's Q7 pair)
                bounce = bounces[rep]
                nc.sync.dma_start(
                    bounce[:].rearrange("(p q) -> p q", p=P), rel16[:]
                )
                nc.gpsimd.memset(wrapped[:], 0)
                wr4 = wrapped[:].rearrange(
                    "p (t b1 b0) -> p t b1 b0", t=S, b1=k, b0=8
                )
                src4 = bass.AP(
                    bounce, 0, [[qp, 16], [k, S], [1, k], [16 * qp, 8]]
                )
                for g in range(2):
                    nc.sync.dma_start(wr4[16 * g : 16 * (g + 1)], src4)

            with (
                tc.tile_pool(name=f"g{rep}", bufs=gbufs) as gp,
                tc.tile_pool(name=f"m{rep}", bufs=mbufs) as mp,
                tc.tile_pool(name=f"r{rep}", bufs=rbufs) as rp,
            ):
                for t in range(S):
                    half = "A" if t < sA else "B"
                    qs = slice(t * k, (t + 1) * k)
                    idxs = wrapped[:, t * k * 8 : (t + 1) * k * 8]

                    g0 = gp.tile([P, k * 2 * c], F32, tag="g0")
                    nc.gpsimd.dma_gather(
                        out_ap=g0[:].rearrange("p (s e) -> p s e", e=2 * c),
                        in_ap=bases[half + "0"], idxs_ap=idxs,
                        num_idxs=P * k, num_idxs_reg=P * k,
                        elem_size=2 * c, elem_step=c, single_packet=False,
                    )
                    g1 = gp.tile([P, k * 2 * c], F32, tag="g1")
                    nc.gpsimd.dma_gather(
                        out_ap=g1[:].rearrange("p (s e) -> p s e", e=2 * c),
                        in_ap=bases[half + "1"], idxs_ap=idxs,
                        num_idxs=P * k, num_idxs_reg=P * k,
                        elem_size=2 * c, elem_step=c, single_packet=False,
                    )

                    # weighted corners; iterate (slot, channel, xcorner)
                    m01 = mp.tile([P, k * 2 * c], F32, tag="m01")
                    m01v = m01[:].rearrange("p (s ch x) -> p s ch x", s=k, ch=c, x=2)
                    g0v = g0[:].rearrange("p (s x ch) -> p s ch x", x=2, ch=c)
                    wAv = wA[:].rearrange("p (q x) -> p q x", x=2)[:, qs, :]
                    wAv = wAv.unsqueeze(2).to_broadcast([P, k, c, 2])
                    nc.vector.tensor_tensor(m01v, g0v, wAv, op=OP.mult)

                    m23 = mp.tile([P, k * 2 * c], F32, tag="m23")
                    m23v = m23[:].rearrange("p (s ch x) -> p s ch x", s=k, ch=c, x=2)
                    g1v = g1[:].rearrange("p (s x ch) -> p s ch x", x=2, ch=c)
                    wBv = wB[:].rearrange("p (q x) -> p q x", x=2)[:, qs, :]
                    wBv = wBv.unsqueeze(2).to_broadcast([P, k, c, 2])
                    nc.vector.tensor_tensor(m23v, g1v, wBv, op=OP.mult)

                    nc.vector.tensor_tensor(m01[:], m01[:], m23[:], op=OP.add)

                    r = rp.tile([P, k * c], F32, tag="r")
                    av = m01[:].rearrange("p (f x) -> p f x", x=2)
                    nc.gpsimd.tensor_tensor(r[:], av[:, :, 0], av[:, :, 1], op=OP.add)

                    nc.sync.dma_start(outdev[:, t * k * c : (t + 1) * k * c], r[:])

    nc.compile()
    return nc


def _host_yhalf(grid_n, h, w):
    """Bit-exact replica of the device's y0 computation -> bottom-half mask."""
    gy = np.asarray(grid_n, np.float32).reshape(-1, 2)[:, 1]
    scy = np.float32((h - 1) / 2.0)
    iy = ((gy + np.float32(1.0)).astype(np.float32) * scy).astype(np.float32)
    y0 = np.floor(iy)
    return y0 >= (h // 2)


def prep_core(x_n, grid_n, sA, sB, h=H, w=W, c=C, k=32):
    """Marshal one sample: slab, binned grid feed, and unpermute metadata."""
    npix = h * w
    S = sA + sB
    qp = S * k
    slot = P * k

    xt = np.zeros((npix + 2 * w, c), np.float32)
    xt[:npix] = np.asarray(x_n, np.float32).reshape(c, npix).T

    grid_flat = np.asarray(grid_n, np.float32).reshape(npix, 2)
    mB = _host_yhalf(grid_n, h, w)
    idsA = np.flatnonzero(~mB)
    idsB = np.flatnonzero(mB)
    assert len(idsA) <= sA * slot and len(idsB) <= sB * slot

    stream = np.empty((S * slot, 2), np.float32)
    stream[:, 0] = -1.0   # synthetic pad: top-left corner pixel (y0 = 0)
    stream[:, 1] = -1.0
    stream[sA * slot :, 1] = 0.9  # bottom-half pad: y0 well into bottom half
    stream[: len(idsA)] = grid_flat[idsA]
    stream[sA * slot : sA * slot + len(idsB)] = grid_flat[idsB]

    # device stream position of each real pixel
    pos = np.empty(npix, np.int64)
    pos[idsA] = np.arange(len(idsA))
    pos[idsB] = sA * slot + np.arange(len(idsB))

    # gridfeed[p, q] = stream[(q//k)*slot + (q%k)*128 + p]
    qq = np.arange(qp)
    ii = (qq // k)[None, :] * slot + (qq % k)[None, :] * P + np.arange(P)[:, None]
    gridfeed = stream[ii].reshape(P, qp * 2)

    # outdev row of stream position i: [p = i%128, q = (i//slot)*k + (i//128)%k]
    prow = pos % P
    qrow = (pos // slot) * k + (pos // P) % k
    rows = prow * qp + qrow
    return {"xt": xt, "gridf": np.ascontiguousarray(gridfeed)}, rows


_PROGRAMS = {}


def get_program(sA, sB, **kw):
    key = (sA, sB, tuple(sorted(kw.items())))
    if key not in _PROGRAMS:
        _PROGRAMS[key] = build_program(sA=sA, sB=sB, **kw)
    return _PROGRAMS[key]


def kernel(x, grid):
    x = np.asarray(x, np.float32)
    grid = np.asarray(grid, np.float32)
    assert x.shape == (N, C, H, W) and grid.shape == (N, H, W, 2)
    k = 32
    slot = P * k

    nB = [int(_host_yhalf(grid[n], H, W).sum()) for n in range(N)]
    sA = max(-(-(H * W - b) // slot) for b in nB)
    sB = max(-(-b // slot) for b in nB)
    nc = get_program(sA, sB)

    in_maps, rowmaps = [], []
    for n in range(N):
        im, rows = prep_core(x[n], grid[n], sA, sB)
        in_maps.append(im)
        rowmaps.append(rows)

    res = bass_utils.run_bass_kernel_spmd(nc, in_maps, core_ids=list(range(N)))

    out = np.empty((N, C, H, W), np.float32)
    for n in range(N):
        od = np.asarray(res.results[n]["outdev"]).reshape(P * (sA + sB) * k, C)
        out[n] = od[rowmaps[n]].T.reshape(C, H, W)
    return out


# revision 14
# speedup vs baseline: 2.4869x; 2.4869x over previous
"""Bilinear grid-sample (align_corners=True, border-clamped) on Trainium2.

Problem: x [8, 64, 256, 256] f32, grid [8, 256, 256, 2] f32 in [-1, 1]
         -> out [8, 64, 256, 256] f32.

Sharding: pure data-parallel over batch N=8 -> one sample per NeuronCore.

Per-core layout strategy (host-side marshalling only; all arithmetic that
is part of the operator runs on-device):
  - x[n] is fed as a "vertically paired" channels-last slab vp [H*W, 4*C]:
    row p holds the full 2x2 bilinear patch anchored at pixel p =
    y0*W + x0, i.e. [x(y0,x0,:), x(y0,x0+1,:), x(y0+1,x0,:),
    x(y0+1,x0+1,:)] (1KB). One output pixel therefore needs exactly ONE
    contiguous 1KB gather read.
  - gathers use the GPSIMD dma_gather custom DMA (int16 indices => 32768
    addressable 1KB rows = half the image). Pixels are binned by their y0
    image half (top: y0 < H/2, bottom: y0 >= H/2); rel = y0*W + x0 -
    (half ? H*W/2 : 0) fits int16 exactly, and the two halves differ only
    in the static base offset of the source access pattern.
  - the host bins pixel ids into [top..., pad, bottom..., pad] where pads
    are synthetic corner pixels; every gather call (slot) is homogeneous
    in half, so its base is static. Binning uses the same fp32 operations
    the device uses, and the device clamps rel to [0, 32767] anyway.
  - the device computes indices/weights, gathers, combines (weighted sum
    of the 4 patch corners), and stores results in stream order; the host
    permutes rows back and transposes to [C, H, W].

dma_gather stream mapping (hardware-verified): stream position i takes
its int16 index from idx_tile[i % 16, i // 16] (partitions 0-15, plus the
replicas at partitions 16-31 for queue 0's second Q7 core) and writes the
gathered row to partition i % 128, slot i // 128. Partition p therefore
owns stream positions == p (mod 128); the grid is fed in that layout, and
a small DRAM bounce rearranges the computed rel16 into the 16-partition
wrapped layout.
"""

import numpy as np

import concourse.bacc as bacc
import concourse.bass as bass
import concourse.tile as tile
from concourse import bass_utils, mybir

N, C, H, W = 8, 64, 256, 256

F32 = mybir.dt.float32
I32 = mybir.dt.int32
I16 = mybir.dt.int16
AF = mybir.ActivationFunctionType
OP = mybir.AluOpType
P = 128


def build_program(
    h=H, w=W, c=C, k=16, sA=17, sB=17, gbufs=3, mbufs=2, rbufs=3, repeat=1,
    do_gather=True, do_combine=True, do_store=True,
):
    """Per-core Bass program for sA top-half + sB bottom-half gather slots.

    Each slot covers 128*k stream positions; the gather fetches one 4*c
    float (1KB) patch row per position from the vertically-paired slab.
    repeat > 1 re-emits the whole pipeline for repeat-delta timing.
    """
    npix = h * w
    halfbase = (h // 2) * w          # patch-row offset of the bottom half
    S = sA + sB
    qp = S * k                        # stream positions per partition
    scx = (w - 1) / 2.0
    scy = (h - 1) / 2.0
    relmax = float(halfbase + w - 1)  # clamp bound; == 32767 at full size
    e4 = 4 * c                        # patch row: 4 corners x c channels

    nc = bacc.Bacc(trn_type="TRN2", debug=False)
    vp = nc.dram_tensor("vp", [npix, e4], F32, kind="ExternalInput")
    gridf = nc.dram_tensor("gridf", [P, qp * 2], F32, kind="ExternalInput")
    bounces = [
        nc.dram_tensor(f"bounce{r}", [P * qp], I16, kind="Internal")
        for r in range(repeat)
    ]
    outdev = nc.dram_tensor("outdev", [P, qp * c], F32, kind="ExternalOutput")

    bases = {
        "A": bass.AP(vp, 0, [[e4, npix], [1, e4]]),
        "B": bass.AP(vp, halfbase * e4, [[e4, npix - halfbase], [1, e4]]),
    }

    with tile.TileContext(nc) as tc:
      for rep in range(repeat):
        with tc.tile_pool(name=f"persist{rep}", bufs=1) as persist:
            w4 = persist.tile([P, qp * 4], F32, tag="w4")  # (w00,w01,w10,w11)
            wrapped = persist.tile([P, qp * 8], I16, tag="wrapped")

            with tc.tile_pool(name=f"prep{rep}", bufs=1) as prep:
                gridt = prep.tile([P, qp * 2], F32, tag="gridt")
                nc.sync.dma_start(gridt[:], gridf[:])
                g3 = gridt[:].rearrange("p (q xy) -> p q xy", xy=2)

                def sc_mul_add(src, scale, name):
                    # (src + 1) * scale with the reference's rounding order,
                    # as two explicitly rounded DVE ops (host replicates it)
                    a = prep.tile([P, qp], F32, tag=f"sma_{name}")
                    nc.vector.tensor_scalar_add(a[:], src, 1.0)
                    nc.vector.tensor_scalar_mul(a[:], a[:], scale)
                    return a

                ix = sc_mul_add(g3[:, :, 0], scx, "x")
                iy = sc_mul_add(g3[:, :, 1], scy, "y")

                def floor_and_fracs(v, name):
                    vi = prep.tile([P, qp], I32, tag=f"vi_{name}")
                    nc.vector.tensor_copy(vi[:], v)
                    vf = prep.tile([P, qp], F32, tag=f"vf_{name}")
                    nc.vector.tensor_copy(vf[:], vi[:])
                    m = prep.tile([P, qp], F32, tag=f"m_{name}")
                    nc.vector.tensor_tensor(m[:], vf[:], v, op=OP.is_gt)
                    nc.vector.tensor_tensor(vf[:], vf[:], m[:], op=OP.subtract)
                    fr1 = prep.tile([P, qp], F32, tag=f"fr1_{name}")
                    nc.vector.tensor_tensor(fr1[:], v, vf[:], op=OP.subtract)
                    fr0 = prep.tile([P, qp], F32, tag=f"fr0_{name}")
                    nc.scalar.activation(fr0[:], fr1[:], AF.Copy, bias=1.0, scale=-1.0)
                    return vf, fr0, fr1

                x0f, wx0, wx1 = floor_and_fracs(ix[:], "x")
                y0f, wy0, wy1 = floor_and_fracs(iy[:], "y")

                w44 = w4[:].rearrange("p (q x) -> p q x", x=4)
                nc.vector.tensor_tensor(w44[:, :, 0], wy0[:], wx0[:], op=OP.mult)
                nc.vector.tensor_tensor(w44[:, :, 1], wy0[:], wx1[:], op=OP.mult)
                nc.vector.tensor_tensor(w44[:, :, 2], wy1[:], wx0[:], op=OP.mult)
                nc.vector.tensor_tensor(w44[:, :, 3], wy1[:], wx1[:], op=OP.mult)

                # rel = y0*w + x0 - halfbase*(y0 >= h/2), clamped to int16 range
                mhalf = prep.tile([P, qp], F32, tag="mhalf")
                nc.vector.tensor_scalar(
                    mhalf[:], y0f[:], float(h // 2), None, op0=OP.is_ge
                )
                idxf = prep.tile([P, qp], F32, tag="idxf")
                nc.vector.scalar_tensor_tensor(
                    idxf[:], in0=y0f[:], scalar=float(w), in1=x0f[:],
                    op0=OP.mult, op1=OP.add,
                )
                relf = prep.tile([P, qp], F32, tag="relf")
                nc.vector.scalar_tensor_tensor(
                    relf[:], in0=mhalf[:], scalar=-float(halfbase), in1=idxf[:],
                    op0=OP.mult, op1=OP.add,
                )
                nc.vector.tensor_scalar_max(relf[:], relf[:], 0.0)
                nc.vector.tensor_scalar_min(relf[:], relf[:], relmax)
                rel16 = prep.tile([P, qp], I16, tag="rel16")
                nc.vector.tensor_copy(rel16[:], relf[:])

                # bounce through DRAM into the 16-partition wrapped layout,
                # replicated to partition groups 0 and 1 (queue 0's Q7 pair)
                bounce = bounces[rep]
                nc.sync.dma_start(
                    bounce[:].rearrange("(p q) -> p q", p=P), rel16[:]
                )
                nc.vector.memset(wrapped[:], 0)
                wr4 = wrapped[:].rearrange(
                    "p (t b1 b0) -> p t b1 b0", t=S, b1=k, b0=8
                )
                src4 = bass.AP(
                    bounce, 0, [[qp, 16], [k, S], [1, k], [16 * qp, 8]]
                )
                for g in range(2):
                    nc.sync.dma_start(wr4[16 * g : 16 * (g + 1)], src4)

            with (
                tc.tile_pool(name=f"g{rep}", bufs=gbufs) as gp,
                tc.tile_pool(name=f"m{rep}", bufs=mbufs) as mp,
                tc.tile_pool(name=f"r{rep}", bufs=rbufs) as rp,
            ):
                for t in range(S):
                    half = "A" if t < sA else "B"
                    qs = slice(t * k, (t + 1) * k)
                    idxs = wrapped[:, t * k * 8 : (t + 1) * k * 8]

                    g0 = gp.tile([P, k * e4], F32, tag="g0")
                    if do_gather:
                        nc.gpsimd.dma_gather(
                            out_ap=g0[:].rearrange("p (s e) -> p s e", e=e4),
                            in_ap=bases[half], idxs_ap=idxs,
                            num_idxs=P * k, num_idxs_reg=P * k,
                            elem_size=e4, single_packet=False,
                        )
                    else:
                        nc.vector.memset(g0[:1, :1], 0)

                    r = rp.tile([P, k * c], F32, tag="r")
                    if do_combine:
                        # weighted corners; iterate (slot, channel, corner)
                        m0 = mp.tile([P, k * e4], F32, tag="m0")
                        m0v = m0[:].rearrange(
                            "p (s ch x) -> p s ch x", s=k, ch=c, x=4
                        )
                        g0v = g0[:].rearrange("p (s x ch) -> p s ch x", x=4, ch=c)
                        w4v = w4[:].rearrange("p (q x) -> p q x", x=4)[:, qs, :]
                        w4v = w4v.unsqueeze(2).to_broadcast([P, k, c, 4])
                        nc.vector.tensor_tensor(m0v, g0v, w4v, op=OP.mult)

                        # y-add: m[..., 0:2] + m[..., 2:4]
                        a = mp.tile([P, k * c * 2], F32, tag="a")
                        m0q = m0[:].rearrange("p (f x) -> p f x", x=2)
                        nc.vector.tensor_tensor(
                            a[:].rearrange("p (f x) -> p f x", x=2),
                            m0q[:, 0::2, :], m0q[:, 1::2, :], op=OP.add,
                        )
                        av = a[:].rearrange("p (f x) -> p f x", x=2)
                        nc.vector.tensor_tensor(
                            r[:], av[:, :, 0], av[:, :, 1], op=OP.add
                        )
                    else:
                        nc.vector.tensor_copy(r[:], g0[:, : k * c])

                    if do_store:
                        nc.sync.dma_start(
                            outdev[:, t * k * c : (t + 1) * k * c], r[:]
                        )

    nc.compile()
    return nc


def _host_yhalf(grid_n, h, w):
    """Bit-exact replica of the device's y0 computation -> bottom-half mask."""
    gy = np.asarray(grid_n, np.float32).reshape(-1, 2)[:, 1]
    scy = np.float32((h - 1) / 2.0)
    iy = ((gy + np.float32(1.0)).astype(np.float32) * scy).astype(np.float32)
    y0 = np.floor(iy)
    return y0 >= (h // 2)


def _build_vp(x_n, h, w, c):
    """Vertically-paired channels-last slab: vp[p] = 2x2 patch at pixel p."""
    npix = h * w
    xt = np.zeros((npix + 2 * w + 2, c), np.float32)
    xt[:npix] = np.asarray(x_n, np.float32).reshape(c, npix).T
    flat = xt.reshape(-1)
    pair = np.lib.stride_tricks.as_strided(
        flat, shape=(npix + w + 1, 2 * c), strides=(4 * c, 4)
    )
    vp = np.empty((npix, 4 * c), np.float32)
    vp[:, : 2 * c] = pair[:npix]
    vp[:, 2 * c :] = pair[w : w + npix]
    return vp


def prep_core(x_n, grid_n, sA, sB, h=H, w=W, c=C, k=16):
    """Marshal one sample: slab, binned grid feed, and unpermute metadata."""
    npix = h * w
    S = sA + sB
    qp = S * k
    slot = P * k

    vp = _build_vp(x_n, h, w, c)

    grid_flat = np.asarray(grid_n, np.float32).reshape(npix, 2)
    mB = _host_yhalf(grid_n, h, w)
    idsA = np.flatnonzero(~mB)
    idsB = np.flatnonzero(mB)
    assert len(idsA) <= sA * slot and len(idsB) <= sB * slot

    stream = np.empty((S * slot, 2), np.float32)
    stream[:, 0] = -1.0   # synthetic pad: top-left corner pixel (y0 = 0)
    stream[:, 1] = -1.0
    stream[sA * slot :, 1] = 0.9  # bottom-half pad: y0 well into bottom half
    stream[: len(idsA)] = grid_flat[idsA]
    stream[sA * slot : sA * slot + len(idsB)] = grid_flat[idsB]

    # device stream position of each real pixel
    pos = np.empty(npix, np.int64)
    pos[idsA] = np.arange(len(idsA))
    pos[idsB] = sA * slot + np.arange(len(idsB))

    # gridfeed[p, q] = stream[(q//k)*slot + (q%k)*128 + p]
    qq = np.arange(qp)
    ii = (qq // k)[None, :] * slot + (qq % k)[None, :] * P + np.arange(P)[:, None]
    gridfeed = stream[ii].reshape(P, qp * 2)

    # outdev row of stream position i: [p = i%128, q = (i//slot)*k + (i//128)%k]
    prow = pos % P
    qrow = (pos // slot) * k + (pos // P) % k
    rows = prow * qp + qrow
    return {"vp": vp, "gridf": np.ascontiguousarray(gridfeed)}, rows


_PROGRAMS = {}


def get_program(sA, sB, **kw):
    key = (sA, sB, tuple(sorted(kw.items())))
    if key not in _PROGRAMS:
        _PROGRAMS[key] = build_program(sA=sA, sB=sB, **kw)
    return _PROGRAMS[key]


KDEF = 16


def kernel(x, grid):
    x = np.asarray(x, np.float32)
    grid = np.asarray(grid, np.float32)
    assert x.shape == (N, C, H, W) and grid.shape == (N, H, W, 2)
    k = KDEF
    slot = P * k

    nB = [int(_host_yhalf(grid[n], H, W).sum()) for n in range(N)]
    sA = max(-(-(H * W - b) // slot) for b in nB)
    sB = max(-(-b // slot) for b in nB)
    nc = get_program(sA, sB)

    in_maps, rowmaps = [], []
    for n in range(N):
        im, rows = prep_core(x[n], grid[n], sA, sB, k=k)
        in_maps.append(im)
        rowmaps.append(rows)

    res = bass_utils.run_bass_kernel_spmd(nc, in_maps, core_ids=list(range(N)))

    out = np.empty((N, C, H, W), np.float32)
    for n in range(N):
        od = np.asarray(res.results[n]["outdev"]).reshape(P * (sA + sB) * k, C)
        out[n] = od[rowmaps[n]].T.reshape(C, H, W)
    return out
